# revision 2
# baseline (speedup 1.0000x reference)
"""Multi-head attention (B=2, S=2048, D=1024, H=16, causal) on 8 TRN2 NeuronCores.

Sharding: core c -> (batch b = c//4, head-group g = c%4, heads 4g..4g+3).
Each core computes Q/K/V projections for its 4 heads, causal flash-style
attention, and a partial output projection (its 256 d_model columns of the
ctx @ wo.T contraction).  Host sums the 4 partials per batch and adds bo.

Device layout convention: "transposed" activations (feature dim on SBUF
partitions, sequence on the free axis) so every matmul contraction runs
along partitions.  Host passes x.T and pre-transposed weight slices so all
DMAs are contiguous.  Matmuls run in float32r (full-rate fp32 storage,
~3e-4 rel err); softmax runs unnormalized with a fused ones-column in the
v projection (zero weights + bias 1) so the ctx matmul also produces the
denominator; normalization happens on DVE afterwards.
"""

import sys

for _p in ("/opt/trn_rl_repo",):
    if _p not in sys.path:
        sys.path.insert(0, _p)

import numpy as np

import concourse.bass as bass
import concourse.mybir as mybir
import concourse.tile as tile
from concourse import bacc, bass_utils

F32 = mybir.dt.float32
F32R = mybir.dt.float32r
AF = mybir.ActivationFunctionType

N_CORES = 8
B, S, D, H = 2, 2048, 1024, 16
HG = 4            # heads per core
DK = 64           # head dim
F = HG * DK       # 256 features per core
FA = HG * (DK + 1)  # 260: v features + per-head ones column (denominator)
SC = S // 512     # 4 q-chunks of 512
DT = D // 128     # 8 d-tiles
FT = F // 128     # 2 f-tiles
ST = S // 128     # 16 s-tiles / k-tiles


def _build():
    nc = bacc.Bacc("TRN2", target_bir_lowering=False, debug=False,
                   num_devices=N_CORES)

    def din(name, shape):
        return nc.dram_tensor(name, shape, F32, kind="ExternalInput").ap()

    xqT = din("xqT", (D, S))
    xkT = din("xkT", (D, S))
    xvT = din("xvT", (D, S))
    wqT = din("wqT", (D, F))
    wkT = din("wkT", (D, F))
    wvT = din("wvT", (D, FA))     # interleaved, with zero ones-columns
    woT = din("woT", (F, D))
    bq8 = din("bq8", (F, 1))
    bk = din("bk", (F, 1))
    bv260 = din("bv260", (128, FA))  # bv broadcast, 1.0 at ones-columns
    outT = nc.dram_tensor("outT", (D, S), F32, kind="ExternalOutput").ap()

    with tile.TileContext(nc) as tc:
        with (
            tc.tile_pool(name="const", bufs=1) as cp,
            tc.tile_pool(name="data", bufs=1) as dp,
            tc.tile_pool(name="io", bufs=1) as iop,
            tc.tile_pool(name="pp", bufs=1, space="PSUM") as pp,
        ):
            # ---- constants / weights -------------------------------------
            wq_t = [cp.tile([128, F], F32R, name=f"wq{d}") for d in range(DT)]
            wk_t = [cp.tile([128, F], F32R, name=f"wk{d}") for d in range(DT)]
            wv_t = [cp.tile([128, FA], F32R, name=f"wv{d}") for d in range(DT)]
            wo_t = [cp.tile([128, D], F32R, name=f"wo{t}") for t in range(FT)]
            for d in range(DT):
                sl = slice(d * 128, (d + 1) * 128)
                nc.sync.dma_start(wq_t[d][:], wqT[sl, :].bitcast(F32R))
                nc.sync.dma_start(wk_t[d][:], wkT[sl, :].bitcast(F32R))
                nc.sync.dma_start(wv_t[d][:], wvT[sl, :].bitcast(F32R))
            for t in range(FT):
                nc.sync.dma_start(wo_t[t][:],
                                  woT[t * 128:(t + 1) * 128, :].bitcast(F32R))
            bq8_t = [cp.tile([128, 1], F32, name=f"bq8{t}") for t in range(FT)]
            bk_t = [cp.tile([128, 1], F32, name=f"bk{t}") for t in range(FT)]
            for t in range(FT):
                nc.sync.dma_start(bq8_t[t][:], bq8[t * 128:(t + 1) * 128, :])
                nc.sync.dma_start(bk_t[t][:], bk[t * 128:(t + 1) * 128, :])
            bv_t = cp.tile([128, FA], F32, name="bv")
            nc.sync.dma_start(bv_t[:], bv260[:])

            # tri01[p, y] = 1 if y >= p else 0  (keep k<=q on diagonal blocks)
            tri = cp.tile([128, 128], F32, name="tri")
            nc.gpsimd.memset(tri[:], 1.0)
            nc.gpsimd.affine_select(
                out=tri[:], in_=tri[:], compare_op=mybir.AluOpType.is_ge,
                fill=0.0, base=0, pattern=[[1, 128]], channel_multiplier=-1)

            # ---- persistent per-core tensors -----------------------------
            qpT = [dp.tile([128, S], F32R, name=f"qpT{t}") for t in range(FT)]
            kpT = [dp.tile([128, S], F32R, name=f"kpT{t}") for t in range(FT)]
            # vp[st]: [128 keys, 4 heads x (64 dv + denominator-ones)]
            vp = [dp.tile([128, FA], F32R, name=f"vp{st}") for st in range(ST)]
            ctxn = [dp.tile([128, S], F32R, name=f"ctxn{t}") for t in range(FT)]

            # ---- phase A: projections ------------------------------------
            # q / k: transposed outputs [f, s] = wT.T @ xT
            for (xT, w_t, qk, scale, bias_t) in (
                    (xqT, wq_t, qpT, 0.125, bq8_t),
                    (xkT, wk_t, kpT, 1.0, bk_t)):
                ptiles = [[pp.tile([128, 512], F32, name=f"pa{t * SC + s}")
                           for s in range(SC)] for t in range(FT)]
                for d in range(DT):
                    xd = iop.tile([128, S], F32R, name="xq", bufs=3)
                    nc.sync.dma_start(
                        xd[:], xT[d * 128:(d + 1) * 128, :].bitcast(F32R))
                    for t in range(FT):
                        lhsT = w_t[d][:, t * 128:(t + 1) * 128]
                        for s in range(SC):
                            nc.tensor.matmul(
                                ptiles[t][s][:],
                                lhsT,
                                xd[:, s * 512:(s + 1) * 512],
                                start=(d == 0), stop=(d == DT - 1))
                for t in range(FT):
                    for s in range(SC):
                        nc.scalar.activation(
                            qk[t][:, s * 512:(s + 1) * 512], ptiles[t][s][:],
                            AF.Identity, bias=bias_t[t][:], scale=scale)

            # v: natural layout [s, f] = xT.T @ wT_aug (+ bias, ones cols)
            for half in range(2):
                pv = [pp.tile([128, 512], F32, name=f"pa{i}")[:, :FA]
                      for i in range(8)]
                for d in range(DT):
                    xd = iop.tile([128, S // 2], F32R, name="xv", bufs=3)
                    nc.sync.dma_start(
                        xd[:], xvT[d * 128:(d + 1) * 128,
                                   half * 1024:(half + 1) * 1024].bitcast(F32R))
                    for s8 in range(8):
                        nc.tensor.matmul(
                            pv[s8][:],
                            xd[:, s8 * 128:(s8 + 1) * 128],
                            wv_t[d][:],
                            start=(d == 0), stop=(d == DT - 1))
                for s8 in range(8):
                    st = half * 8 + s8
                    nc.vector.tensor_add(vp[st][:], pv[s8][:], bv_t[:])

            # ---- phase B: attention --------------------------------------
            sc_i = 0
            for h in range(HG):
                t, off = h // 2, 64 * (h % 2)
                for j in range(SC):
                    pc = pp.tile([128, 512], F32, name=f"pa{3 + (h * SC + j) % 2}")
                    # k-tiles descending: diagonals (narrow) first, full-width
                    # kt=0 last carries stop=True over every psum column.
                    kts = list(range(4 * j + 3, -1, -1))
                    for kt in kts:
                        r = kt - 4 * j
                        c0 = 128 * r if r > 0 else 0
                        w = 512 - c0
                        psc = pp.tile([128, 512], F32,
                                      name=f"pa{sc_i % 3}")[:, :w]
                        sc_i += 1
                        nc.tensor.matmul(
                            psc,
                            kpT[t][off:off + 64, kt * 128:(kt + 1) * 128],
                            qpT[t][off:off + 64, j * 512 + c0:(j + 1) * 512],
                            start=True, stop=True)
                        ex = iop.tile([128, 512], F32R, name="ex", bufs=4)[:, :w]
                        nc.scalar.activation(ex, psc, AF.Exp)
                        if r >= 0:
                            nc.vector.tensor_mul(ex[:, 0:128], ex[:, 0:128],
                                                 tri[:])
                        nc.tensor.matmul(
                            pc[0:65, c0:512],
                            vp[kt][:, 65 * h:65 * h + 65],
                            ex,
                            start=(kt == kts[0]), stop=(kt == 0))
                    rc = iop.tile([1, 512], F32, name="rc", bufs=2)
                    nc.vector.reciprocal(rc[:], pc[64:65, :])
                    bc = iop.tile([64, 512], F32, name="bc", bufs=2)
                    nc.gpsimd.partition_broadcast(bc[:], rc[:])
                    nc.vector.tensor_mul(
                        ctxn[t][off:off + 64, j * 512:(j + 1) * 512],
                        pc[0:64, :], bc[:])

            # ---- phase C: output projection ------------------------------
            for e in range(DT):
                for s in range(SC):
                    po = pp.tile([128, 512], F32,
                                 name=f"pa{5 + (e * SC + s) % 3}")
                    for t in range(FT):
                        nc.tensor.matmul(
                            po[:],
                            wo_t[t][:, e * 128:(e + 1) * 128],
                            ctxn[t][:, s * 512:(s + 1) * 512],
                            start=(t == 0), stop=(t == FT - 1))
                    ob = iop.tile([128, 512], F32, name="ob", bufs=3)
                    nc.vector.tensor_copy(ob[:], po[:])
                    nc.sync.dma_start(
                        outT[e * 128:(e + 1) * 128, s * 512:(s + 1) * 512],
                        ob[:])

    nc.compile()
    return nc


_NC_CACHE = {}


def _get_nc():
    if "nc" not in _NC_CACHE:
        _NC_CACHE["nc"] = _build()
    return _NC_CACHE["nc"]


def _in_maps(q, k, v, wq, bq, wk, bk, wv, bv, wo):
    maps = []
    xT = {}
    for b in range(B):
        xT[b] = tuple(np.ascontiguousarray(x[b].T) for x in (q, k, v))
    per_g = {}
    for g in range(HG):
        sl = slice(g * F, (g + 1) * F)
        # interleave v weights/bias with the denominator ones-column per head
        wv_aug = np.zeros((D, FA), np.float32)
        bv_aug = np.zeros((FA,), np.float32)
        wv_sl = wv[sl, :]          # (256, 1024)
        bv_sl = bv[sl]
        for h in range(HG):
            wv_aug[:, h * 65:h * 65 + 64] = wv_sl[h * 64:(h + 1) * 64, :].T
            bv_aug[h * 65:h * 65 + 64] = bv_sl[h * 64:(h + 1) * 64]
            bv_aug[h * 65 + 64] = 1.0
        per_g[g] = dict(
            wqT=np.ascontiguousarray(wq[sl, :].T),
            wkT=np.ascontiguousarray(wk[sl, :].T),
            wvT=wv_aug,
            woT=np.ascontiguousarray(wo[:, sl].T),
            bq8=np.ascontiguousarray((bq[sl] / 8.0).reshape(F, 1)),
            bk=np.ascontiguousarray(bk[sl].reshape(F, 1)),
            bv260=np.ascontiguousarray(np.broadcast_to(bv_aug, (128, FA))),
        )
    for c in range(N_CORES):
        b, g = c // HG, c % HG
        m = dict(xqT=xT[b][0], xkT=xT[b][1], xvT=xT[b][2])
        m.update(per_g[g])
        maps.append(m)
    return maps


def run(inputs, trace=False, tmpdir=None):
    nc = _get_nc()
    q = np.asarray(inputs["q"], np.float32)
    k = np.asarray(inputs["k"], np.float32)
    v = np.asarray(inputs["v"], np.float32)
    maps = _in_maps(q, k, v,
                    np.asarray(inputs["wq"], np.float32),
                    np.asarray(inputs["bq"], np.float32),
                    np.asarray(inputs["wk"], np.float32),
                    np.asarray(inputs["bk"], np.float32),
                    np.asarray(inputs["wv"], np.float32),
                    np.asarray(inputs["bv"], np.float32),
                    np.asarray(inputs["wo"], np.float32))
    kwargs = {}
    if trace:
        kwargs = dict(trace=True, tmpdir=tmpdir)
    res = bass_utils.run_bass_kernel_spmd(
        nc, maps, core_ids=list(range(N_CORES)), **kwargs)
    bo = np.asarray(inputs["bo"], np.float32)
    out = np.empty((B, S, D), np.float32)
    for b in range(B):
        acc = res.results[4 * b]["outT"].copy()
        for g in range(1, HG):
            acc += res.results[4 * b + g]["outT"]
        out[b] = acc.T + bo
    return out, res


def kernel(**inputs):
    out, _ = run(inputs)
    return out
